# revision 6
# baseline (speedup 1.0000x reference)
"""NearAggregator Trainium2 Bass kernel — TensorE-centric redesign.

Math (per batch item b):
    Kcat   = concat([near_emb, delta_xy, delta_cs], -1)          # [N, 132]
    scores = (Kcat @ W_key + b_key) . B_query[b] / sqrt(64)      # [N]
    out[b] = softmax(scores) @ near_emb[b]                       # [128]

Reformulation (same algebra as the previous DVE-based kernel):
  * Fold W_key into the query: qp[b] = 0.125 * (W_key^T @ B_query[b]),
    so scores[b,n] = near[b,n,:].qp[b,:128] + sc4[b,n] where sc4 is the
    tiny host-precomputed delta contribution.  b_key is softmax-invariant.

Why TensorE: DVE's fused mult+reduce (scalar_tensor_tensor) runs at 1x
with no perf modes -> ~194ns per neighbor column; with the scale pass on
ScalarE (~300ns/op) the old kernel was engine-bound at ~378us while DMA
needed only ~180us.  Both data passes move to the PE array instead:

  * scores, per item: one matmul with the item's near slice as the
    STATIONARY operand (lhsT = nearT[d,128n], fp8e3m4) and the item's
    projected query as a 1-column MOVING operand -> psum column [N,1].
    128 items fill a [N,G] psum tile with NO diagonal extraction.
  * pooling, per item: symmetric — stationary = near[n,128d] (bf16),
    moving = exp-weights column [N,1] -> psum column = pooled^T [D,1].
  * measured on hw (probe): 55.6 ns per LDW+MM pair end-to-end,
    ~35 ns/pair of PE-active time — LDWEIGHTS overlaps matmuls via the
    PE's 64-deep reorder window.

Precision: near is streamed twice — fp8e3m4 (4 mantissa bits, ~1.8%
elementwise) for the scores pass where error only perturbs softmax
weights (~1% output effect), bf16 for the value/pooling pass (~0.4%).
Total ~1.2% fro vs the 2e-2 gate, and HBM traffic drops 64MB -> 49MB
per core.

Softmax: scores land [n-part, item-free]; exp needs no max-subtraction
(scores ~ N(0,0.58)).  sumexp = ones-stationary matmul over the n
partitions.  Normalisation (pooled/sumexp) happens on host (<0.1% of
FLOPs) because recip is free-dim-indexed in this layout and a partition
broadcast is impossible on DVE.

Pipeline: pooling of group g is deferred one iteration so its e-weights
(DVE add + ScalarE exp) are ready — the PE alternates scores(g) /
pool(g-1) without stalling.  Input tiles double/triple-buffered; DMA
issue is split across the two HWDGE engines (sync + scalar).

Data parallel over 8 NeuronCores: batch 8192 -> 1024 per core.
"""

import numpy as np

B = 8192
N = 128
D = 128
CORES = 8
PB = B // CORES            # 1024 items per core
G = 64                     # items per group (= psum free dim)
NGROUPS = PB // G          # 16

_NC = None


def _build():
    import concourse.tile as tile
    from concourse import bacc, mybir

    f32 = mybir.dt.float32
    bf16 = mybir.dt.bfloat16
    fp8 = mybir.dt.float8e3
    add = mybir.AluOpType.add
    bypass = mybir.AluOpType.bypass

    nc = bacc.Bacc(
        "TRN2",
        target_bir_lowering=False,
        debug=False,
        enable_asserts=True,
        num_devices=CORES,
    )
    npt = nc.dram_tensor("npt", [N, PB, D], bf16, kind="ExternalInput").ap()
    dpt = nc.dram_tensor("dpt", [D, PB, N], fp8, kind="ExternalInput").ap()
    qpt = nc.dram_tensor("qpt", [D, PB], bf16, kind="ExternalInput").ap()
    sc4t = nc.dram_tensor("sc4t", [N, PB], bf16, kind="ExternalInput").ap()
    pout = nc.dram_tensor("pout", [D, PB], f32, kind="ExternalOutput").ap()
    seout = nc.dram_tensor("seout", [1, PB], f32, kind="ExternalOutput").ap()
    ones_dram = nc.inline_tensor(np.ones((N, 1), dtype=np.float32), name="ones").ap()

    with tile.TileContext(nc) as tc:
        from contextlib import ExitStack

        ctx = ExitStack()
        with ctx:
            consts = ctx.enter_context(tc.tile_pool(name="consts", bufs=1))
            npp = ctx.enter_context(tc.tile_pool(name="npp", bufs=8))
            dpp = ctx.enter_context(tc.tile_pool(name="dpp", bufs=6))
            qpp = ctx.enter_context(tc.tile_pool(name="qpp", bufs=4))
            s4p = ctx.enter_context(tc.tile_pool(name="s4p", bufs=4))
            epp = ctx.enter_context(tc.tile_pool(name="epp", bufs=2))
            ebp = ctx.enter_context(tc.tile_pool(name="ebp", bufs=3))
            osb = ctx.enter_context(tc.tile_pool(name="osb", bufs=3))
            psc = ctx.enter_context(tc.tile_pool(name="psc", bufs=2, space="PSUM"))
            ppl = ctx.enter_context(tc.tile_pool(name="ppl", bufs=2, space="PSUM"))
            pse = ctx.enter_context(tc.tile_pool(name="pse", bufs=2, space="PSUM"))

            ones_f = consts.tile([N, 1], f32)
            nc.sync.dma_start(ones_f[:], ones_dram[:])
            ones_bf = consts.tile([N, 1], bf16)
            nc.scalar.copy(ones_bf[:], ones_f[:])

            def emit_loads(g):
                # scores inputs first (they gate the PE), values after
                b0 = g * G
                dp_t = dpp.tile([D, G, N], fp8, name=f"dp{g}", tag="dp")
                for k in range(2):
                    s = slice(k * (G // 2), (k + 1) * (G // 2))
                    nc.scalar.dma_start(dp_t[:, s, :], dpt[:, b0 + k * (G // 2) : b0 + (k + 1) * (G // 2), :])
                qp_t = qpp.tile([D, G], bf16, tag="qp")
                nc.gpsimd.dma_start(qp_t[:], qpt[:, b0 : b0 + G])
                s4_t = s4p.tile([N, G], bf16, tag="s4")
                nc.gpsimd.dma_start(s4_t[:], sc4t[:, b0 : b0 + G])
                np_t = npp.tile([N, G, D], bf16, name=f"np{g}", tag="np")
                # per-partition contiguous 16KB; split across queues
                for k in range(4):
                    s = slice(k * (G // 4), (k + 1) * (G // 4))
                    nc.sync.dma_start(np_t[:, s, :], npt[:, b0 + k * (G // 4) : b0 + (k + 1) * (G // 4), :])
                return np_t, dp_t, qp_t, s4_t

            def emit_pool(np_t, e_bf, g):
                b0 = g * G
                pl_ps = ppl.tile([D, G], f32, tag="pl")
                se_ps = pse.tile([1, G], f32, tag="se")
                nc.tensor.matmul(
                    se_ps[:], ones_bf[:], e_bf[:], start=True, stop=True,
                    skip_group_check=True,
                )
                for i in range(G):
                    nc.tensor.matmul(
                        pl_ps[:, i : i + 1],
                        np_t[:, i, :],
                        e_bf[:, i : i + 1],
                        start=True,
                        stop=True,
                        skip_group_check=True,
                    )
                se_sb = osb.tile([1, G], f32, tag="sesb")
                nc.vector.tensor_copy(se_sb[:], se_ps[:])
                nc.gpsimd.dma_start(seout[:, b0 : b0 + G], se_sb[:])
                pl_sb = osb.tile([D, G], f32, tag="plsb")
                nc.vector.tensor_copy(pl_sb[:], pl_ps[:])
                nc.gpsimd.dma_start(pout[:, b0 : b0 + G], pl_sb[:])

            loads = emit_loads(0)
            pending = None
            for g in range(NGROUPS):
                np_t, dp_t, qp_t, s4_t = loads
                if g + 1 < NGROUPS:
                    loads = emit_loads(g + 1)

                sc_ps = psc.tile([N, G], f32, tag="sc")
                for i in range(G):
                    nc.tensor.matmul(
                        sc_ps[:, i : i + 1],
                        dp_t[:, i, :],
                        qp_t[:, i : i + 1],
                        start=True,
                        stop=True,
                        skip_group_check=True,
                    )
                # e_pre = scores + sc4 ; e = exp(e_pre) in bf16
                e_pre = epp.tile([N, G], f32, tag="epre")
                nc.vector.scalar_tensor_tensor(
                    out=e_pre[:], in0=sc_ps[:], scalar=1.0, in1=s4_t[:],
                    op0=bypass, op1=add,
                )
                e_bf = ebp.tile([N, G], bf16, tag="ebf")
                nc.scalar.activation(
                    e_bf[:], e_pre[:], func=mybir.ActivationFunctionType.Exp
                )

                if pending is not None:
                    emit_pool(*pending)
                pending = (np_t, e_bf, g)

            emit_pool(*pending)

    nc.compile()
    return nc


def _get_nc():
    global _NC
    if _NC is None:
        _NC = _build()
    return _NC


def prepare_in_maps(near_emb, delta_xy, delta_cs, B_query, W_key):
    """Host-side reformulation: fold W into the query, precompute the
    delta score term, and lay near out in the two PE-friendly layouts."""
    import ml_dtypes

    bf16 = ml_dtypes.bfloat16
    fp8 = ml_dtypes.float8_e3m4

    near_emb = np.asarray(near_emb, dtype=np.float32)
    delta_xy = np.asarray(delta_xy, dtype=np.float32)
    delta_cs = np.asarray(delta_cs, dtype=np.float32)
    B_query = np.asarray(B_query, dtype=np.float32)
    W_key = np.asarray(W_key, dtype=np.float32)

    qp = 0.125 * (B_query @ W_key.T)          # [B, 132]
    sc4 = (
        delta_xy[:, :, 0] * qp[:, 128:129]
        + delta_xy[:, :, 1] * qp[:, 129:130]
        + delta_cs[:, :, 0] * qp[:, 130:131]
        + delta_cs[:, :, 1] * qp[:, 131:132]
    )                                          # [B, N]

    in_maps = []
    for c in range(CORES):
        s = slice(c * PB, (c + 1) * PB)
        nb = near_emb[s]                                   # [PB, N, D]
        nbf = nb.astype(bf16)
        nf8 = nb.astype(fp8)
        in_maps.append(
            {
                "npt": np.ascontiguousarray(nbf.transpose(1, 0, 2)),   # [N, PB, D]
                "dpt": np.ascontiguousarray(nf8.transpose(2, 0, 1)),   # [D, PB, N]
                "qpt": np.ascontiguousarray(qp[s, :128].T).astype(bf16),
                "sc4t": np.ascontiguousarray(sc4[s].T).astype(bf16),
            }
        )
    return in_maps


def finalize(results):
    """Host epilogue: transpose pooled^T back and normalise by sumexp."""
    outs = []
    for c in range(CORES):
        poolT = np.asarray(results[c]["pout"], dtype=np.float32)   # [D, PB]
        se = np.asarray(results[c]["seout"], dtype=np.float32)     # [1, PB]
        outs.append(poolT.T / se.T)
    return np.concatenate(outs, axis=0)


def kernel(near_emb, delta_xy, delta_cs, B_query, W_key, b_key=None, **_ignored):
    from concourse import bass_utils

    in_maps = prepare_in_maps(near_emb, delta_xy, delta_cs, B_query, W_key)
    nc = _get_nc()
    res = bass_utils.run_bass_kernel_spmd(nc, in_maps, core_ids=list(range(CORES)))
    return finalize(res.results)


# revision 11
# speedup vs baseline: 1.3021x; 1.3021x over previous
"""NearAggregator Trainium2 Bass kernel — TensorE-centric redesign.

Math (per batch item b):
    Kcat   = concat([near_emb, delta_xy, delta_cs], -1)          # [N, 132]
    scores = (Kcat @ W_key + b_key) . B_query[b] / sqrt(64)      # [N]
    out[b] = softmax(scores) @ near_emb[b]                       # [128]

Reformulation (same algebra as the previous DVE-based kernel):
  * Fold W_key into the query: qp[b] = 0.125 * (W_key^T @ B_query[b]),
    so scores[b,n] = near[b,n,:].qp[b,:128] + sc4[b,n] where sc4 is the
    tiny host-precomputed delta contribution.  b_key is softmax-invariant.

Why TensorE: DVE's fused mult+reduce (scalar_tensor_tensor) runs at 1x
with no perf modes -> ~194ns per neighbor column; with the scale pass on
ScalarE (~300ns/op) the old kernel was engine-bound at ~378us while DMA
needed only ~180us.  Both data passes move to the PE array instead:

  * scores, per item: one matmul with the item's near slice as the
    STATIONARY operand (lhsT = nearT[d,128n], fp8e3m4) and the item's
    projected query as a 1-column MOVING operand -> psum column [N,1].
    128 items fill a [N,G] psum tile with NO diagonal extraction.
  * pooling, per item: symmetric — stationary = near[n,128d] (bf16),
    moving = exp-weights column [N,1] -> psum column = pooled^T [D,1].
  * measured on hw (probe): 55.6 ns per LDW+MM pair end-to-end,
    ~35 ns/pair of PE-active time — LDWEIGHTS overlaps matmuls via the
    PE's 64-deep reorder window.

Precision: near is streamed twice — fp8e3m4 (4 mantissa bits, ~1.8%
elementwise) for the scores pass where error only perturbs softmax
weights (~1% output effect), bf16 for the value/pooling pass (~0.4%).
Total ~1.2% fro vs the 2e-2 gate, and HBM traffic drops 64MB -> 49MB
per core.

Softmax: scores land [n-part, item-free]; exp needs no max-subtraction
(scores ~ N(0,0.58)).  sumexp = ones-stationary matmul over the n
partitions.  Normalisation (pooled/sumexp) happens on host (<0.1% of
FLOPs) because recip is free-dim-indexed in this layout and a partition
broadcast is impossible on DVE.

Pipeline: pooling of group g is deferred one iteration so its e-weights
(DVE add + ScalarE exp) are ready — the PE alternates scores(g) /
pool(g-1) without stalling.  Input tiles double/triple-buffered; DMA
issue is split across the two HWDGE engines (sync + scalar).

Data parallel over 8 NeuronCores: batch 8192 -> 1024 per core.
"""

import numpy as np

B = 8192
N = 128
D = 128
CORES = 8
PB = B // CORES            # 1024 items per core
G = 128                    # items per group (= psum free dim)
NGROUPS = PB // G          # 8

_NC = None


def _build():
    import concourse.tile as tile
    from concourse import bacc, mybir

    f32 = mybir.dt.float32
    bf16 = mybir.dt.bfloat16
    fp8 = mybir.dt.float8e3
    add = mybir.AluOpType.add
    bypass = mybir.AluOpType.bypass

    nc = bacc.Bacc(
        "TRN2",
        target_bir_lowering=False,
        debug=False,
        enable_asserts=True,
        num_devices=CORES,
    )
    npt = nc.dram_tensor("npt", [N, PB, D], fp8, kind="ExternalInput").ap()
    dpt = nc.dram_tensor("dpt", [D, PB, N], fp8, kind="ExternalInput").ap()
    qpt = nc.dram_tensor("qpt", [D, PB], bf16, kind="ExternalInput").ap()
    sc4t = nc.dram_tensor("sc4t", [N, PB], bf16, kind="ExternalInput").ap()
    pout = nc.dram_tensor("pout", [D, PB], f32, kind="ExternalOutput").ap()
    seout = nc.dram_tensor("seout", [1, PB], f32, kind="ExternalOutput").ap()
    ones_dram = nc.inline_tensor(np.ones((N, 1), dtype=np.float32), name="ones").ap()

    with tile.TileContext(nc) as tc:
        from contextlib import ExitStack

        ctx = ExitStack()
        with ctx:
            consts = ctx.enter_context(tc.tile_pool(name="consts", bufs=1))
            npp = ctx.enter_context(tc.tile_pool(name="npp", bufs=4))
            dpp = ctx.enter_context(tc.tile_pool(name="dpp", bufs=4))
            qpp = ctx.enter_context(tc.tile_pool(name="qpp", bufs=4))
            s4p = ctx.enter_context(tc.tile_pool(name="s4p", bufs=4))
            epp = ctx.enter_context(tc.tile_pool(name="epp", bufs=2))
            ebp = ctx.enter_context(tc.tile_pool(name="ebp", bufs=3))
            osb = ctx.enter_context(tc.tile_pool(name="osb", bufs=3))
            psc = ctx.enter_context(tc.tile_pool(name="psc", bufs=2, space="PSUM"))
            ppl = ctx.enter_context(tc.tile_pool(name="ppl", bufs=2, space="PSUM"))
            pse = ctx.enter_context(tc.tile_pool(name="pse", bufs=2, space="PSUM"))

            ones_f = consts.tile([N, 1], f32)
            nc.sync.dma_start(ones_f[:], ones_dram[:])
            ones_bf = consts.tile([N, 1], bf16)
            nc.scalar.copy(ones_bf[:], ones_f[:])

            def emit_loads(g):
                # scores inputs first (they gate the PE), values after
                b0 = g * G
                dp_t = dpp.tile([D, G, N], fp8, name=f"dp{g}", tag="dp")
                for k in range(4):
                    s = slice(k * (G // 4), (k + 1) * (G // 4))
                    nc.scalar.dma_start(dp_t[:, s, :], dpt[:, b0 + k * (G // 4) : b0 + (k + 1) * (G // 4), :])
                qp_t = qpp.tile([D, G], bf16, tag="qp")
                nc.gpsimd.dma_start(qp_t[:], qpt[:, b0 : b0 + G])
                s4_t = s4p.tile([N, G], bf16, tag="s4")
                nc.gpsimd.dma_start(s4_t[:], sc4t[:, b0 : b0 + G])
                np_t = npp.tile([N, G, D], fp8, name=f"np{g}", tag="np")
                # per-partition contiguous 16KB; split across queues
                for k in range(4):
                    s = slice(k * (G // 4), (k + 1) * (G // 4))
                    nc.sync.dma_start(np_t[:, s, :], npt[:, b0 + k * (G // 4) : b0 + (k + 1) * (G // 4), :])
                return np_t, dp_t, qp_t, s4_t

            def emit_pool(np_t, e_bf, g):
                b0 = g * G
                pl_ps = ppl.tile([D, G], f32, tag="pl")
                se_ps = pse.tile([1, G], f32, tag="se")
                nc.tensor.matmul(
                    se_ps[:], ones_bf[:], e_bf[:], start=True, stop=True,
                    skip_group_check=True,
                )
                for i in range(G):
                    nc.tensor.matmul(
                        pl_ps[:, i : i + 1],
                        np_t[:, i, :],
                        e_bf[:, i : i + 1],
                        start=True,
                        stop=True,
                        skip_group_check=True,
                    )
                se_sb = osb.tile([1, G], f32, tag="sesb")
                nc.vector.tensor_copy(se_sb[:], se_ps[:])
                nc.gpsimd.dma_start(seout[:, b0 : b0 + G], se_sb[:])
                pl_sb = osb.tile([D, G], f32, tag="plsb")
                nc.vector.tensor_copy(pl_sb[:], pl_ps[:])
                nc.gpsimd.dma_start(pout[:, b0 : b0 + G], pl_sb[:])

            loads = emit_loads(0)
            pending = None
            for g in range(NGROUPS):
                np_t, dp_t, qp_t, s4_t = loads
                if g + 1 < NGROUPS:
                    loads = emit_loads(g + 1)

                sc_ps = psc.tile([N, G], f32, tag="sc")
                for i in range(G):
                    nc.tensor.matmul(
                        sc_ps[:, i : i + 1],
                        dp_t[:, i, :],
                        qp_t[:, i : i + 1],
                        start=True,
                        stop=True,
                        skip_group_check=True,
                    )
                # e_pre = scores + sc4 ; e = exp(e_pre) in bf16
                e_pre = epp.tile([N, G], f32, tag="epre")
                nc.vector.scalar_tensor_tensor(
                    out=e_pre[:], in0=sc_ps[:], scalar=1.0, in1=s4_t[:],
                    op0=bypass, op1=add,
                )
                e_bf = ebp.tile([N, G], bf16, tag="ebf")
                nc.scalar.activation(
                    e_bf[:], e_pre[:], func=mybir.ActivationFunctionType.Exp
                )

                if pending is not None:
                    emit_pool(*pending)
                pending = (np_t, e_bf, g)

            emit_pool(*pending)

    nc.compile()
    return nc


def _get_nc():
    global _NC
    if _NC is None:
        _NC = _build()
    return _NC


def prepare_in_maps(near_emb, delta_xy, delta_cs, B_query, W_key):
    """Host-side reformulation: fold W into the query, precompute the
    delta score term, and lay near out in the two PE-friendly layouts."""
    import ml_dtypes

    bf16 = ml_dtypes.bfloat16
    fp8 = ml_dtypes.float8_e3m4

    near_emb = np.asarray(near_emb, dtype=np.float32)
    delta_xy = np.asarray(delta_xy, dtype=np.float32)
    delta_cs = np.asarray(delta_cs, dtype=np.float32)
    B_query = np.asarray(B_query, dtype=np.float32)
    W_key = np.asarray(W_key, dtype=np.float32)

    qp = 0.125 * (B_query @ W_key.T)          # [B, 132]
    sc4 = (
        delta_xy[:, :, 0] * qp[:, 128:129]
        + delta_xy[:, :, 1] * qp[:, 129:130]
        + delta_cs[:, :, 0] * qp[:, 130:131]
        + delta_cs[:, :, 1] * qp[:, 131:132]
    )                                          # [B, N]

    in_maps = []
    for c in range(CORES):
        s = slice(c * PB, (c + 1) * PB)
        nb = near_emb[s]                                   # [PB, N, D]
        nf8 = nb.astype(fp8)
        in_maps.append(
            {
                "npt": np.ascontiguousarray(nf8.transpose(1, 0, 2)),   # [N, PB, D]
                "dpt": np.ascontiguousarray(nf8.transpose(2, 0, 1)),   # [D, PB, N]
                "qpt": np.ascontiguousarray(qp[s, :128].T).astype(bf16),
                "sc4t": np.ascontiguousarray(sc4[s].T).astype(bf16),
            }
        )
    return in_maps


def finalize(results):
    """Host epilogue: transpose pooled^T back and normalise by sumexp."""
    outs = []
    for c in range(CORES):
        poolT = np.asarray(results[c]["pout"], dtype=np.float32)   # [D, PB]
        se = np.asarray(results[c]["seout"], dtype=np.float32)     # [1, PB]
        outs.append(poolT.T / se.T)
    return np.concatenate(outs, axis=0)


def kernel(near_emb, delta_xy, delta_cs, B_query, W_key, b_key=None, **_ignored):
    from concourse import bass_utils

    in_maps = prepare_in_maps(near_emb, delta_xy, delta_cs, B_query, W_key)
    nc = _get_nc()
    res = bass_utils.run_bass_kernel_spmd(nc, in_maps, core_ids=list(range(CORES)))
    return finalize(res.results)
